# revision 35
# baseline (speedup 1.0000x reference)
"""DGM retrieval-knn kernel for Trainium2 (8 NeuronCores, Bass/Tile).

Computes, for x:[4,4096,64], W:[64,64], temperature:[1], q:[4,4096,4096]:
  x_emb = (x @ W)                                  [4,4096,64]
  D[b,i,j] = ||x_emb_i - x_emb_j||^2
  lq = D*exp(clip(T,-5,5)) - log(-log(q+eps))
  logprobs, indices = top_k(-lq, 10)
  edges = global src/tgt index list                 [2, 4*4096*10]
returns (x_emb, edges, logprobs) like the reference.

Sharding: 8 cores = 4 batch elements x 2 row-halves (2048 rows each).
Each core handles the full j range (4096) for its rows, so the per-row
top-k is complete on a single core. All per-core programs are identical
(SPMD): per-core data differences come only through the input tensors.

Device algorithm per core (16 row-tiles of 128 rows x 4096 cols):
  - PE: PSUM = (-s*D) via one contract-66 matmul per 512-chunk: lhsT rows
    = [2s*embT_own; -s; -s*x2_own], rhs rows = [embT_all; x2_all; 1].
  - ACT: g = ln(-ln(q+eps)) in place (2 passes), plus PSUM->SBUF eviction
    of -s*D in half-row ops.
  - Pool: neg_lq = g + (-s*D)  (tensor_add, SBUF only).
  - DVE: per 256-wide chunk (16 of them) max8 + max_index -> 128 candidates
    with in-chunk indices; then top-16 of the candidates (max8, max_index,
    match_replace, max8, max_index).
Engine balance per core (cost model): DVE ~207us (bottleneck), ACT ~177us,
PE ~135us, Pool ~150us, DMA ~110us; total ~264us.
Host: gathers global indices with one take_along_axis, builds edges.

The per-row top-10 is exact unless >=9 of a row's true top-10 fall in a
single 256-wide chunk (P ~ 2e-9 per row with uniform Gumbel noise).
"""

import numpy as np

from concourse import bacc, bass, mybir, tile
from concourse.bass_utils import run_bass_kernel_spmd
from concourse.masks import make_identity

F32 = mybir.dt.float32
U32 = mybir.dt.uint32
AF = mybir.ActivationFunctionType

B, N, D, K = 4, 4096, 64, 10
RPC = N // 2          # rows per core
NT = RPC // 128       # row tiles per core
NCH = 16              # stage-1 chunks per row
CHW = N // NCH        # chunk width (256)
NCAND = NCH * 8       # candidates per row (128)
EPS = 1e-8
NEG_INF = -3.0e38


def build_module(skip=()):
    skip = set(skip)
    nc = bacc.Bacc(None, target_bir_lowering=False, debug=False)
    xa_d = nc.declare_dram_parameter("xT_all", [D, N], F32, isOutput=False)
    xo_d = nc.declare_dram_parameter("xT_own", [D, RPC], F32, isOutput=False)
    w_d = nc.declare_dram_parameter("w", [D, D], F32, isOutput=False)
    cs_d = nc.declare_dram_parameter("consts", [128, 2], F32, isOutput=False)
    q_d = nc.declare_dram_parameter("q", [RPC, N], F32, isOutput=False)
    ones_d = nc.declare_dram_parameter("ones_row", [1, N], F32, isOutput=False)
    negs_d = nc.declare_dram_parameter("negs_row", [1, RPC], F32, isOutput=False)
    xe_d = nc.declare_dram_parameter("x_emb", [RPC, D], F32, isOutput=True)
    v16_d = nc.declare_dram_parameter("v16", [RPC, 16], F32, isOutput=True)
    p16_d = nc.declare_dram_parameter("p16", [RPC, 16], U32, isOutput=True)
    cli_d = nc.declare_dram_parameter("cli", [RPC, NCAND], U32, isOutput=True)

    with tile.TileContext(nc) as tc:
        with tc.tile_pool(name="bank", bufs=1) as bank:
            L66 = bank.tile([D + 2, RPC], F32)   # lhsT bank (own rows)
            B66 = bank.tile([D + 2, N], F32)     # rhs bank (all rows)
            I128 = bank.tile([128, 128], F32)
            Wt = bank.tile([D, D], F32)
            CS = bank.tile([128, 2], F32)        # col0 = 2s, col1 = -s
            ones_col = bank.tile([D, 1], F32)
            eps_ap = bank.tile([128, 1], F32)

            make_identity(nc, I128)
            nc.sync.dma_start(Wt[:], w_d[:])
            nc.sync.dma_start(CS[:], cs_d[:])
            nc.gpsimd.memset(ones_col[:], 1.0)
            nc.gpsimd.memset(eps_ap[:], EPS)

            qio_cm = tc.tile_pool(name="qio", bufs=4)
            qio = qio_cm.__enter__()

            # ---------------- phase A: banks + x_emb output ----------------
            with (
                tc.tile_pool(name="pa_ps", bufs=6, space="PSUM") as pa_ps,
                tc.tile_pool(name="pa_sb", bufs=4) as pa_sb,
                tc.tile_pool(name="pa_scr", bufs=1) as pa_scr,
            ):
                ETo = pa_scr.tile([D, RPC], F32)   # embT of own rows
                xT = pa_scr.tile([D, N], F32)
                xoT = pa_scr.tile([D, RPC], F32)
                SQa = pa_scr.tile([D, N], F32)
                SQo = pa_scr.tile([D, RPC], F32)
                nc.sync.dma_start(L66[D:D + 1, :], negs_d[:])
                nc.sync.dma_start(B66[D + 1:D + 2, :], ones_d[:])
                nc.sync.dma_start(xT[:], xa_d[:])
                nc.sync.dma_start(xoT[:], xo_d[:])

                # q prefetch: overlap the first tiles' DMA + ln passes with
                # phase A (engine streams are in-order; emitted here so ACT
                # has work while the banks are built by PE/DVE/Pool).
                PREF = 3
                pref_q = {}
                for t in range(PREF):
                    Qt = qio.tile([128, N], F32, tag="Qt")
                    nc.sync.dma_start(Qt[:], q_d[128 * t:128 * (t + 1), :])
                    if "ln" not in skip:
                        nc.scalar.activation(Qt[:], Qt[:], AF.Ln, bias=eps_ap[:],
                                             scale=1.0)
                        nc.scalar.activation(Qt[:], Qt[:], AF.Ln, bias=0.0,
                                             scale=-1.0)
                    pref_q[t] = Qt

                x2s = bank.tile([1, RPC], F32)
                for c in range(N // 512):
                    # --- x_all chunk: embT, square, x2 row ---
                    ps_e = pa_ps.tile([D, 512], F32, tag="pa")
                    nc.tensor.matmul(ps_e[:], Wt[:], xT[:, 512 * c:512 * (c + 1)],
                                     start=True, stop=True)
                    nc.vector.tensor_copy(B66[0:D, 512 * c:512 * (c + 1)], ps_e[:])
                    nc.gpsimd.tensor_mul(SQa[:, 512 * c:512 * (c + 1)],
                                         B66[0:D, 512 * c:512 * (c + 1)],
                                         B66[0:D, 512 * c:512 * (c + 1)])
                    ps_s = pa_ps.tile([1, 512], F32, tag="pa")
                    nc.tensor.matmul(ps_s[:], ones_col[:],
                                     SQa[:, 512 * c:512 * (c + 1)],
                                     start=True, stop=True)
                    nc.vector.tensor_copy(B66[D:D + 1, 512 * c:512 * (c + 1)],
                                          ps_s[:])
                    if c >= RPC // 512:
                        continue
                    # --- x_own chunk: embT, L rows, square, x2 row, x_emb out ---
                    ps_eo = pa_ps.tile([D, 512], F32, tag="pa")
                    nc.tensor.matmul(ps_eo[:], Wt[:], xoT[:, 512 * c:512 * (c + 1)],
                                     start=True, stop=True)
                    nc.vector.tensor_copy(ETo[:, 512 * c:512 * (c + 1)], ps_eo[:])
                    nc.gpsimd.tensor_scalar_mul(L66[0:D, 512 * c:512 * (c + 1)],
                                                ETo[:, 512 * c:512 * (c + 1)],
                                                CS[0:D, 0:1])
                    nc.gpsimd.tensor_mul(SQo[:, 512 * c:512 * (c + 1)],
                                         ETo[:, 512 * c:512 * (c + 1)],
                                         ETo[:, 512 * c:512 * (c + 1)])
                    ps_s2 = pa_ps.tile([1, 512], F32, tag="pa")
                    nc.tensor.matmul(ps_s2[:], ones_col[:],
                                     SQo[:, 512 * c:512 * (c + 1)],
                                     start=True, stop=True)
                    nc.vector.tensor_scalar_mul(x2s[0:1, 512 * c:512 * (c + 1)],
                                                ps_s2[:], CS[0:1, 1:2])
                    nc.sync.dma_start(L66[D + 1:D + 2, 512 * c:512 * (c + 1)],
                                      x2s[0:1, 512 * c:512 * (c + 1)])
                    for tt in range(4 * c, 4 * (c + 1)):
                        ps_xe = pa_ps.tile([128, D], F32, tag="pa")
                        nc.tensor.transpose(ps_xe[:], ETo[:, 128 * tt:128 * (tt + 1)],
                                            I128[0:D, 0:D])
                        xe_t = pa_sb.tile([128, D], F32)
                        nc.vector.tensor_copy(xe_t[:], ps_xe[:])
                        nc.sync.dma_start(xe_d[128 * tt:128 * (tt + 1), :], xe_t[:])

            # ---------------- phase B: gumbel + topk ----------------
            with (
                tc.tile_pool(name="ps", bufs=2, space="PSUM") as psp,
                tc.tile_pool(name="sdp", bufs=3) as sdp,
                tc.tile_pool(name="nlp", bufs=4) as nlp,
                tc.tile_pool(name="outp", bufs=3) as outp,
            ):
                for t in range(NT):
                    if t in pref_q:
                        Qt = pref_q[t]
                    else:
                        Qt = qio.tile([128, N], F32, tag="Qt")
                        nc.sync.dma_start(Qt[:], q_d[128 * t:128 * (t + 1), :])
                        # Qt <- ln(q + eps); Qt <- g = ln(-Qt)  (in place)
                        if "ln" not in skip:
                            nc.scalar.activation(Qt[:], Qt[:], AF.Ln,
                                                 bias=eps_ap[:], scale=1.0)
                            nc.scalar.activation(Qt[:], Qt[:], AF.Ln, bias=0.0,
                                                 scale=-1.0)

                    NLt = nlp.tile([128, N], F32)
                    Ct = outp.tile([128, NCAND], F32)
                    CLIt = outp.tile([128, NCAND], U32)
                    V16t = outp.tile([128, 16], F32)
                    P16t = outp.tile([128, 16], U32)
                    Lsl = L66[:, 128 * t:128 * (t + 1)]
                    for h in range(2):
                        SDt2 = sdp.tile([128, N // 2], F32, tag="SDt2")
                        psh = psp.tile([128, N // 2], F32)
                        if "mm" not in skip:
                            for c in range(4):
                                ch = 4 * h + c
                                nc.tensor.matmul(psh[:, 512 * c:512 * (c + 1)], Lsl,
                                                 B66[:, 512 * ch:512 * (ch + 1)],
                                                 start=True, stop=True)
                        else:
                            nc.vector.memset(psh[:], 0.0)
                        # ACT evicts raw -s*D; Pool combines with g
                        nc.scalar.mul(SDt2[:], psh[:], 1.0)
                        nc.gpsimd.tensor_add(
                            out=NLt[:, 2048 * h:2048 * (h + 1)],
                            in0=Qt[:, 2048 * h:2048 * (h + 1)],
                            in1=SDt2[:])

                    if "topk" in skip:
                        nc.vector.max(out=V16t[:, 0:8], in_=NLt[:])
                        nc.vector.max(out=V16t[:, 8:16], in_=NLt[:])
                        nc.vector.tensor_copy(P16t[:], V16t[:].bitcast(U32))
                        nc.vector.tensor_copy(CLIt[:], Ct[:].bitcast(U32))
                    else:
                        for c in range(NCH):
                            sl = NLt[:, CHW * c:CHW * (c + 1)]
                            nc.vector.max(out=Ct[:, 8 * c:8 * (c + 1)], in_=sl)
                            nc.vector.max_index(out=CLIt[:, 8 * c:8 * (c + 1)],
                                                in_max=Ct[:, 8 * c:8 * (c + 1)],
                                                in_values=sl)
                        nc.vector.max(out=V16t[:, 0:8], in_=Ct[:])
                        nc.vector.max_index(out=P16t[:, 0:8], in_max=V16t[:, 0:8],
                                            in_values=Ct[:])
                        Cm = outp.tile([128, NCAND], F32)
                        nc.vector.match_replace(out=Cm[:], in_to_replace=V16t[:, 0:8],
                                                in_values=Ct[:], imm_value=NEG_INF)
                        nc.vector.max(out=V16t[:, 8:16], in_=Cm[:])
                        nc.vector.max_index(out=P16t[:, 8:16], in_max=V16t[:, 8:16],
                                            in_values=Cm[:])

                    nc.sync.dma_start(v16_d[128 * t:128 * (t + 1), :], V16t[:])
                    nc.sync.dma_start(p16_d[128 * t:128 * (t + 1), :], P16t[:])
                    nc.sync.dma_start(cli_d[128 * t:128 * (t + 1), :], CLIt[:])
            qio_cm.__exit__(None, None, None)
    nc.finalize()
    return nc


def make_in_maps(x, W, temperature, q):
    x = np.ascontiguousarray(x, np.float32)
    W = np.ascontiguousarray(W, np.float32)
    q = np.ascontiguousarray(q, np.float32)
    s = np.exp(np.clip(np.asarray(temperature, np.float32), -5.0, 5.0))[0]
    consts = np.zeros([128, 2], np.float32)
    consts[:, 0] = np.float32(2.0) * s
    consts[:, 1] = -s
    in_maps = []
    for core in range(2 * B):
        bb, h = core // 2, core % 2
        r0 = h * RPC
        xt = np.ascontiguousarray(x[bb].T)
        in_maps.append({
            "xT_all": xt,
            "xT_own": np.ascontiguousarray(xt[:, r0:r0 + RPC]),
            "w": W,
            "consts": consts,
            "ones_row": np.ones([1, N], np.float32),
            "negs_row": np.full([1, RPC], -s, np.float32),
            "q": np.ascontiguousarray(q[bb, r0:r0 + RPC, :]),
        })
    return in_maps


def postprocess(results):
    """results: list of 8 dicts with x_emb/v16/p16/cli -> (x_emb, edges, logprobs)."""
    x_emb = np.empty([B, N, D], np.float32)
    logprobs = np.empty([B, N, K], np.float32)
    indices = np.empty([B, N, K], np.int32)
    chunk_base = (np.arange(NCAND, dtype=np.int64) // 8) * CHW
    for core in range(2 * B):
        r = results[core]
        bb, h = core // 2, core % 2
        r0 = h * RPC
        x_emb[bb, r0:r0 + RPC] = r["x_emb"]
        cj = r["cli"].astype(np.int64) + chunk_base[None, :]
        j16 = np.take_along_axis(cj, r["p16"].astype(np.int64), axis=1)
        logprobs[bb, r0:r0 + RPC] = r["v16"][:, :K]
        indices[bb, r0:r0 + RPC] = j16[:, :K].astype(np.int32)
    off = (np.arange(B, dtype=np.int32) * N)[:, None, None]
    src = np.broadcast_to(np.arange(N, dtype=np.int32)[None, :, None],
                          (B, N, K)) + off
    tgt = indices + off
    edges = np.stack([src.reshape(-1), tgt.reshape(-1)], axis=0)
    return x_emb, edges, logprobs


_module_cache = {}


def kernel(x, W, temperature, q):
    if "nc" not in _module_cache:
        _module_cache["nc"] = build_module()
    nc = _module_cache["nc"]
    in_maps = make_in_maps(x, W, temperature, q)
    res = run_bass_kernel_spmd(nc, in_maps, list(range(2 * B)))
    return postprocess(res.results)


# revision 42
# speedup vs baseline: 1.0047x; 1.0047x over previous
"""DGM retrieval-knn kernel for Trainium2 (8 NeuronCores, Bass/Tile).

Computes, for x:[4,4096,64], W:[64,64], temperature:[1], q:[4,4096,4096]:
  x_emb = (x @ W)                                  [4,4096,64]
  D[b,i,j] = ||x_emb_i - x_emb_j||^2
  lq = D*exp(clip(T,-5,5)) - log(-log(q+eps))
  logprobs, indices = top_k(-lq, 10)
  edges = global src/tgt index list                 [2, 4*4096*10]
returns (x_emb, edges, logprobs) like the reference.

Sharding: 8 cores = 4 batch elements x 2 row-halves (2048 rows each).
Each core handles the full j range (4096) for its rows, so the per-row
top-k is complete on a single core. All per-core programs are identical
(SPMD): per-core data differences come only through the input tensors.

Device algorithm per core (16 row-tiles of 128 rows x 4096 cols):
  - PE: PSUM = (-s*D) via one contract-66 matmul per 512-chunk: lhsT rows
    = [2s*embT_own; -s; -s*x2_own], rhs rows = [embT_all; x2_all; 1].
  - ACT: g = ln(-ln(q+eps)) in place (2 passes), plus PSUM->SBUF eviction
    of -s*D in half-row ops.
  - Pool: neg_lq = g + (-s*D)  (tensor_add, SBUF only).
  - DVE: per 256-wide chunk (16 of them) max8 + max_index -> 128 candidates
    with in-chunk indices; then top-16 of the candidates (max8, max_index,
    match_replace, max8, max_index).
Engine balance per core (cost model): DVE ~207us (bottleneck), ACT ~177us,
PE ~135us, Pool ~150us, DMA ~110us; total ~264us.
Host: gathers global indices with one take_along_axis, builds edges.

The per-row top-10 is exact unless >=9 of a row's true top-10 fall in a
single 256-wide chunk (P ~ 2e-9 per row with uniform Gumbel noise).
"""

import numpy as np

from concourse import bacc, bass, mybir, tile
from concourse.bass_utils import run_bass_kernel_spmd
from concourse.masks import make_identity

F32 = mybir.dt.float32
U32 = mybir.dt.uint32
AF = mybir.ActivationFunctionType

B, N, D, K = 4, 4096, 64, 10
RPC = N // 2          # rows per core
NT = RPC // 128       # row tiles per core
NCH = 16              # stage-1 chunks per row
CHW = N // NCH        # chunk width (256)
NCAND = NCH * 8       # candidates per row (128)
EPS = 1e-8
NEG_INF = -3.0e38


def build_module(skip=()):
    skip = set(skip)
    nc = bacc.Bacc(None, target_bir_lowering=False, debug=False)
    xa_d = nc.declare_dram_parameter("xT_all", [D, N], F32, isOutput=False)
    xo_d = nc.declare_dram_parameter("xT_own", [D, RPC], F32, isOutput=False)
    w_d = nc.declare_dram_parameter("w", [D, D], F32, isOutput=False)
    cs_d = nc.declare_dram_parameter("consts", [128, 2], F32, isOutput=False)
    q_d = nc.declare_dram_parameter("q", [RPC, N], F32, isOutput=False)
    ones_d = nc.declare_dram_parameter("ones_row", [1, N], F32, isOutput=False)
    negs_d = nc.declare_dram_parameter("negs_row", [1, RPC], F32, isOutput=False)
    xe_d = nc.declare_dram_parameter("x_emb", [RPC, D], F32, isOutput=True)
    v16_d = nc.declare_dram_parameter("v16", [RPC, 16], F32, isOutput=True)
    p16_d = nc.declare_dram_parameter("p16", [RPC, 16], U32, isOutput=True)
    cli_d = nc.declare_dram_parameter("cli", [RPC, NCAND], U32, isOutput=True)

    with tile.TileContext(nc) as tc:
        with tc.tile_pool(name="bank", bufs=1) as bank:
            L66 = bank.tile([D + 2, RPC], F32)   # lhsT bank (own rows)
            B66 = bank.tile([D + 2, N], F32)     # rhs bank (all rows)
            I128 = bank.tile([128, 128], F32)
            Wt = bank.tile([D, D], F32)
            CS = bank.tile([128, 2], F32)        # col0 = 2s, col1 = -s
            ones_col = bank.tile([D, 1], F32)
            eps_ap = bank.tile([128, 1], F32)

            make_identity(nc, I128)
            nc.sync.dma_start(Wt[:], w_d[:])
            nc.sync.dma_start(CS[:], cs_d[:])
            nc.gpsimd.memset(ones_col[:], 1.0)
            nc.gpsimd.memset(eps_ap[:], EPS)

            qio_cm = tc.tile_pool(name="qio", bufs=4)
            qio = qio_cm.__enter__()

            # ---------------- phase A: banks + x_emb output ----------------
            with (
                tc.tile_pool(name="pa_ps", bufs=8, space="PSUM") as pa_ps,
                tc.tile_pool(name="pa_sb", bufs=4) as pa_sb,
                tc.tile_pool(name="pa_scr", bufs=1) as pa_scr,
            ):
                ETo = pa_scr.tile([D, RPC], F32)   # embT of own rows
                xT = pa_scr.tile([D, N], F32)
                xoT = pa_scr.tile([D, RPC], F32)
                SQa = pa_scr.tile([D, N], F32)
                SQo = pa_scr.tile([D, RPC], F32)
                nc.sync.dma_start(L66[D:D + 1, :], negs_d[:])
                nc.sync.dma_start(B66[D + 1:D + 2, :], ones_d[:])
                nc.sync.dma_start(xT[:], xa_d[:])
                nc.sync.dma_start(xoT[:], xo_d[:])

                # q prefetch: overlap the first tiles' DMA + ln passes with
                # phase A (engine streams are in-order; emitted here so ACT
                # has work while the banks are built by PE/DVE/Pool).
                PREF = 2
                pref_q = {}
                for t in range(PREF):
                    Qt = qio.tile([128, N], F32, tag="Qt")
                    nc.sync.dma_start(Qt[:], q_d[128 * t:128 * (t + 1), :])
                    if "ln" not in skip:
                        nc.scalar.activation(Qt[:], Qt[:], AF.Ln, bias=eps_ap[:],
                                             scale=1.0)
                        nc.scalar.activation(Qt[:], Qt[:], AF.Ln, bias=0.0,
                                             scale=-1.0)
                    pref_q[t] = Qt

                x2s = bank.tile([1, RPC], F32)
                for c in range(N // 512):
                    # --- x_all chunk: embT, square, x2 row ---
                    ps_e = pa_ps.tile([D, 512], F32, tag="pa")
                    nc.tensor.matmul(ps_e[:], Wt[:], xT[:, 512 * c:512 * (c + 1)],
                                     start=True, stop=True)
                    nc.vector.tensor_copy(B66[0:D, 512 * c:512 * (c + 1)], ps_e[:])
                    nc.gpsimd.tensor_mul(SQa[:, 512 * c:512 * (c + 1)],
                                         B66[0:D, 512 * c:512 * (c + 1)],
                                         B66[0:D, 512 * c:512 * (c + 1)])
                    ps_s = pa_ps.tile([1, 512], F32, tag="pa")
                    nc.tensor.matmul(ps_s[:], ones_col[:],
                                     SQa[:, 512 * c:512 * (c + 1)],
                                     start=True, stop=True)
                    nc.vector.tensor_copy(B66[D:D + 1, 512 * c:512 * (c + 1)],
                                          ps_s[:])
                    if c >= RPC // 512:
                        continue
                    # --- x_own chunk: embT, L rows, square, x2 row, x_emb out ---
                    ps_eo = pa_ps.tile([D, 512], F32, tag="pa")
                    nc.tensor.matmul(ps_eo[:], Wt[:], xoT[:, 512 * c:512 * (c + 1)],
                                     start=True, stop=True)
                    nc.vector.tensor_copy(ETo[:, 512 * c:512 * (c + 1)], ps_eo[:])
                    nc.gpsimd.tensor_scalar_mul(L66[0:D, 512 * c:512 * (c + 1)],
                                                ETo[:, 512 * c:512 * (c + 1)],
                                                CS[0:D, 0:1])
                    nc.gpsimd.tensor_mul(SQo[:, 512 * c:512 * (c + 1)],
                                         ETo[:, 512 * c:512 * (c + 1)],
                                         ETo[:, 512 * c:512 * (c + 1)])
                    ps_s2 = pa_ps.tile([1, 512], F32, tag="pa")
                    nc.tensor.matmul(ps_s2[:], ones_col[:],
                                     SQo[:, 512 * c:512 * (c + 1)],
                                     start=True, stop=True)
                    nc.vector.tensor_scalar_mul(x2s[0:1, 512 * c:512 * (c + 1)],
                                                ps_s2[:], CS[0:1, 1:2])
                    nc.sync.dma_start(L66[D + 1:D + 2, 512 * c:512 * (c + 1)],
                                      x2s[0:1, 512 * c:512 * (c + 1)])
                    for tt in range(4 * c, 4 * (c + 1)):
                        ps_xe = pa_ps.tile([128, D], F32, tag="pa")
                        nc.tensor.transpose(ps_xe[:], ETo[:, 128 * tt:128 * (tt + 1)],
                                            I128[0:D, 0:D])
                        xe_t = pa_sb.tile([128, D], F32)
                        nc.vector.tensor_copy(xe_t[:], ps_xe[:])
                        nc.sync.dma_start(xe_d[128 * tt:128 * (tt + 1), :], xe_t[:])

            # ---------------- phase B: gumbel + topk ----------------
            with (
                tc.tile_pool(name="ps", bufs=2, space="PSUM") as psp,
                tc.tile_pool(name="sdp", bufs=3) as sdp,
                tc.tile_pool(name="nlp", bufs=4) as nlp,
                tc.tile_pool(name="outp", bufs=3) as outp,
            ):
                for t in range(NT):
                    if t in pref_q:
                        Qt = pref_q[t]
                    else:
                        Qt = qio.tile([128, N], F32, tag="Qt")
                        nc.sync.dma_start(Qt[:], q_d[128 * t:128 * (t + 1), :])
                        # Qt <- ln(q + eps); Qt <- g = ln(-Qt)  (in place)
                        if "ln" not in skip:
                            nc.scalar.activation(Qt[:], Qt[:], AF.Ln,
                                                 bias=eps_ap[:], scale=1.0)
                            nc.scalar.activation(Qt[:], Qt[:], AF.Ln, bias=0.0,
                                                 scale=-1.0)

                    NLt = nlp.tile([128, N], F32)
                    Ct = outp.tile([128, NCAND], F32)
                    CLIt = outp.tile([128, NCAND], U32)
                    V16t = outp.tile([128, 16], F32)
                    P16t = outp.tile([128, 16], U32)
                    Lsl = L66[:, 128 * t:128 * (t + 1)]
                    for h in range(2):
                        SDt2 = sdp.tile([128, N // 2], F32, tag="SDt2")
                        psh = psp.tile([128, N // 2], F32)
                        if "mm" not in skip:
                            for c in range(4):
                                ch = 4 * h + c
                                nc.tensor.matmul(psh[:, 512 * c:512 * (c + 1)], Lsl,
                                                 B66[:, 512 * ch:512 * (ch + 1)],
                                                 start=True, stop=True)
                        else:
                            nc.vector.memset(psh[:], 0.0)
                        # ACT evicts raw -s*D; Pool combines with g
                        nc.scalar.mul(SDt2[:], psh[:], 1.0)
                        nc.gpsimd.tensor_add(
                            out=NLt[:, 2048 * h:2048 * (h + 1)],
                            in0=Qt[:, 2048 * h:2048 * (h + 1)],
                            in1=SDt2[:])

                    if "topk" in skip:
                        nc.vector.max(out=V16t[:, 0:8], in_=NLt[:])
                        nc.vector.max(out=V16t[:, 8:16], in_=NLt[:])
                        nc.vector.tensor_copy(P16t[:], V16t[:].bitcast(U32))
                        nc.vector.tensor_copy(CLIt[:], Ct[:].bitcast(U32))
                    else:
                        for c in range(NCH):
                            sl = NLt[:, CHW * c:CHW * (c + 1)]
                            nc.vector.max(out=Ct[:, 8 * c:8 * (c + 1)], in_=sl)
                            nc.vector.max_index(out=CLIt[:, 8 * c:8 * (c + 1)],
                                                in_max=Ct[:, 8 * c:8 * (c + 1)],
                                                in_values=sl)
                        nc.vector.max(out=V16t[:, 0:8], in_=Ct[:])
                        nc.vector.max_index(out=P16t[:, 0:8], in_max=V16t[:, 0:8],
                                            in_values=Ct[:])
                        Cm = outp.tile([128, NCAND], F32)
                        nc.vector.match_replace(out=Cm[:], in_to_replace=V16t[:, 0:8],
                                                in_values=Ct[:], imm_value=NEG_INF)
                        nc.vector.max(out=V16t[:, 8:16], in_=Cm[:])
                        nc.vector.max_index(out=P16t[:, 8:16], in_max=V16t[:, 8:16],
                                            in_values=Cm[:])

                    nc.sync.dma_start(v16_d[128 * t:128 * (t + 1), :], V16t[:])
                    nc.sync.dma_start(p16_d[128 * t:128 * (t + 1), :], P16t[:])
                    nc.sync.dma_start(cli_d[128 * t:128 * (t + 1), :], CLIt[:])
            qio_cm.__exit__(None, None, None)
    nc.finalize()
    return nc


def make_in_maps(x, W, temperature, q):
    x = np.ascontiguousarray(x, np.float32)
    W = np.ascontiguousarray(W, np.float32)
    q = np.ascontiguousarray(q, np.float32)
    s = np.exp(np.clip(np.asarray(temperature, np.float32), -5.0, 5.0))[0]
    consts = np.zeros([128, 2], np.float32)
    consts[:, 0] = np.float32(2.0) * s
    consts[:, 1] = -s
    in_maps = []
    for core in range(2 * B):
        bb, h = core // 2, core % 2
        r0 = h * RPC
        xt = np.ascontiguousarray(x[bb].T)
        in_maps.append({
            "xT_all": xt,
            "xT_own": np.ascontiguousarray(xt[:, r0:r0 + RPC]),
            "w": W,
            "consts": consts,
            "ones_row": np.ones([1, N], np.float32),
            "negs_row": np.full([1, RPC], -s, np.float32),
            "q": np.ascontiguousarray(q[bb, r0:r0 + RPC, :]),
        })
    return in_maps


def postprocess(results):
    """results: list of 8 dicts with x_emb/v16/p16/cli -> (x_emb, edges, logprobs)."""
    x_emb = np.empty([B, N, D], np.float32)
    logprobs = np.empty([B, N, K], np.float32)
    indices = np.empty([B, N, K], np.int32)
    chunk_base = (np.arange(NCAND, dtype=np.int64) // 8) * CHW
    for core in range(2 * B):
        r = results[core]
        bb, h = core // 2, core % 2
        r0 = h * RPC
        x_emb[bb, r0:r0 + RPC] = r["x_emb"]
        cj = r["cli"].astype(np.int64) + chunk_base[None, :]
        j16 = np.take_along_axis(cj, r["p16"].astype(np.int64), axis=1)
        logprobs[bb, r0:r0 + RPC] = r["v16"][:, :K]
        indices[bb, r0:r0 + RPC] = j16[:, :K].astype(np.int32)
    off = (np.arange(B, dtype=np.int32) * N)[:, None, None]
    src = np.broadcast_to(np.arange(N, dtype=np.int32)[None, :, None],
                          (B, N, K)) + off
    tgt = indices + off
    edges = np.stack([src.reshape(-1), tgt.reshape(-1)], axis=0)
    return x_emb, edges, logprobs


_module_cache = {}


def kernel(x, W, temperature, q):
    if "nc" not in _module_cache:
        _module_cache["nc"] = build_module()
    nc = _module_cache["nc"]
    in_maps = make_in_maps(x, W, temperature, q)
    res = run_bass_kernel_spmd(nc, in_maps, list(range(2 * B)))
    return postprocess(res.results)


# revision 47
# speedup vs baseline: 1.0271x; 1.0223x over previous
"""DGM retrieval-knn kernel for Trainium2 (8 NeuronCores, Bass/Tile).

Computes, for x:[4,4096,64], W:[64,64], temperature:[1], q:[4,4096,4096]:
  x_emb = (x @ W)                                  [4,4096,64]
  D[b,i,j] = ||x_emb_i - x_emb_j||^2
  lq = D*exp(clip(T,-5,5)) - log(-log(q+eps))
  logprobs, indices = top_k(-lq, 10)
  edges = global src/tgt index list                 [2, 4*4096*10]
returns (x_emb, edges, logprobs) like the reference.

Sharding: 8 cores = 4 batch elements x 2 row-halves (2048 rows each).
Each core handles the full j range (4096) for its rows, so the per-row
top-k is complete on a single core. All per-core programs are identical
(SPMD): per-core data differences come only through the input tensors.

Device algorithm per core (16 row-tiles of 128 rows x 4096 cols):
  - PE: PSUM = (-s*D) via one contract-66 matmul per 512-chunk: lhsT rows
    = [2s*embT_own; -s; -s*x2_own], rhs rows = [embT_all; x2_all; 1].
  - ACT: g = ln(-ln(q+eps)) in place (2 passes), plus PSUM->SBUF eviction
    of -s*D in half-row ops.
  - Pool: neg_lq = g + (-s*D)  (tensor_add, SBUF only).
  - DVE: per 256-wide chunk (16 of them) max8 + max_index -> 128 candidates
    with in-chunk indices; then top-16 of the candidates (max8, max_index,
    match_replace, max8, max_index).
Engine balance per core (cost model): DVE ~207us (bottleneck), ACT ~177us,
PE ~135us, Pool ~150us, DMA ~110us; total ~264us.
Host: gathers global indices with one take_along_axis, builds edges.

The per-row top-10 is exact unless >=9 of a row's true top-10 fall in a
single 256-wide chunk (P ~ 2e-9 per row with uniform Gumbel noise).
"""

import numpy as np

from concourse import bacc, bass, mybir, tile
from concourse.bass_utils import run_bass_kernel_spmd
from concourse.masks import make_identity

F32 = mybir.dt.float32
U32 = mybir.dt.uint32
AF = mybir.ActivationFunctionType

B, N, D, K = 4, 4096, 64, 10
RPC = N // 2          # rows per core
NT = RPC // 128       # row tiles per core
NCH = 16              # stage-1 chunks per row
CHW = N // NCH        # chunk width (256)
NCAND = NCH * 8       # candidates per row (128)
EPS = 1e-8
NEG_INF = -3.0e38


def build_module(skip=()):
    skip = set(skip)
    nc = bacc.Bacc(None, target_bir_lowering=False, debug=False)
    xa_d = nc.declare_dram_parameter("xT_all", [D, N], F32, isOutput=False)
    xo_d = nc.declare_dram_parameter("xT_own", [D, RPC], F32, isOutput=False)
    w_d = nc.declare_dram_parameter("w", [D, D], F32, isOutput=False)
    cs_d = nc.declare_dram_parameter("consts", [128, 2], F32, isOutput=False)
    q_d = nc.declare_dram_parameter("q", [RPC, N], F32, isOutput=False)
    ones_d = nc.declare_dram_parameter("ones_row", [1, N], F32, isOutput=False)
    negs_d = nc.declare_dram_parameter("negs_row", [1, RPC], F32, isOutput=False)
    xe_d = nc.declare_dram_parameter("x_emb", [RPC, D], F32, isOutput=True)
    v16_d = nc.declare_dram_parameter("v16", [RPC, 16], F32, isOutput=True)
    p16_d = nc.declare_dram_parameter("p16", [RPC, 16], U32, isOutput=True)
    cli_d = nc.declare_dram_parameter("cli", [RPC, NCAND], U32, isOutput=True)

    with tile.TileContext(nc) as tc:
        with tc.tile_pool(name="bank", bufs=1) as bank:
            L66 = bank.tile([D + 2, RPC], F32)   # lhsT bank (own rows)
            B66 = bank.tile([D + 2, N], F32)     # rhs bank (all rows)
            I128 = bank.tile([128, 128], F32)
            Wt = bank.tile([D, D], F32)
            CS = bank.tile([128, 2], F32)        # col0 = 2s, col1 = -s
            ones_col = bank.tile([D, 1], F32)
            eps_ap = bank.tile([128, 1], F32)

            make_identity(nc, I128)
            nc.sync.dma_start(Wt[:], w_d[:])
            nc.sync.dma_start(CS[:], cs_d[:])
            nc.gpsimd.memset(ones_col[:], 1.0)
            nc.gpsimd.memset(eps_ap[:], EPS)

            qio_cm = tc.tile_pool(name="qio", bufs=4)
            qio = qio_cm.__enter__()

            # ---------------- phase A: banks + x_emb output ----------------
            with (
                tc.tile_pool(name="pa_ps", bufs=8, space="PSUM") as pa_ps,
                tc.tile_pool(name="pa_sb", bufs=4) as pa_sb,
                tc.tile_pool(name="pa_scr", bufs=1) as pa_scr,
            ):
                ETo = pa_scr.tile([D, RPC], F32)   # embT of own rows
                xT = pa_scr.tile([D, N], F32)
                xoT = pa_scr.tile([D, RPC], F32)
                SQa = pa_scr.tile([D, N], F32)
                SQo = pa_scr.tile([D, RPC], F32)
                nc.sync.dma_start(L66[D:D + 1, :], negs_d[:])
                nc.sync.dma_start(B66[D + 1:D + 2, :], ones_d[:])
                nc.sync.dma_start(xT[:], xa_d[:])
                nc.sync.dma_start(xoT[:], xo_d[:])

                # q prefetch: overlap the first tiles' DMA + ln passes with
                # phase A (engine streams are in-order; emitted here so ACT
                # has work while the banks are built by PE/DVE/Pool).
                PREF = 2
                pref_q = {}
                for t in range(PREF):
                    Qt = qio.tile([128, N], F32, tag="Qt")
                    nc.sync.dma_start(Qt[:], q_d[128 * t:128 * (t + 1), :])
                    if "ln" not in skip:
                        nc.scalar.activation(Qt[:], Qt[:], AF.Ln, bias=eps_ap[:],
                                             scale=1.0)
                        nc.scalar.activation(Qt[:], Qt[:], AF.Ln, bias=0.0,
                                             scale=-1.0)
                    pref_q[t] = Qt

                x2s = bank.tile([1, RPC], F32)
                for c in range(N // 512):
                    # --- x_all chunk: embT, square, x2 row ---
                    ps_e = pa_ps.tile([D, 512], F32, tag="pa")
                    nc.tensor.matmul(ps_e[:], Wt[:], xT[:, 512 * c:512 * (c + 1)],
                                     start=True, stop=True)
                    nc.vector.tensor_copy(B66[0:D, 512 * c:512 * (c + 1)], ps_e[:])
                    nc.gpsimd.tensor_mul(SQa[:, 512 * c:512 * (c + 1)],
                                         B66[0:D, 512 * c:512 * (c + 1)],
                                         B66[0:D, 512 * c:512 * (c + 1)])
                    ps_s = pa_ps.tile([1, 512], F32, tag="pa")
                    nc.tensor.matmul(ps_s[:], ones_col[:],
                                     SQa[:, 512 * c:512 * (c + 1)],
                                     start=True, stop=True)
                    nc.vector.tensor_copy(B66[D:D + 1, 512 * c:512 * (c + 1)],
                                          ps_s[:])
                    if c >= RPC // 512:
                        continue
                    # --- x_own chunk: embT, L rows, square, x2 row, x_emb out ---
                    ps_eo = pa_ps.tile([D, 512], F32, tag="pa")
                    nc.tensor.matmul(ps_eo[:], Wt[:], xoT[:, 512 * c:512 * (c + 1)],
                                     start=True, stop=True)
                    nc.vector.tensor_copy(ETo[:, 512 * c:512 * (c + 1)], ps_eo[:])
                    nc.gpsimd.tensor_scalar_mul(L66[0:D, 512 * c:512 * (c + 1)],
                                                ETo[:, 512 * c:512 * (c + 1)],
                                                CS[0:D, 0:1])
                    nc.gpsimd.tensor_mul(SQo[:, 512 * c:512 * (c + 1)],
                                         ETo[:, 512 * c:512 * (c + 1)],
                                         ETo[:, 512 * c:512 * (c + 1)])
                    ps_s2 = pa_ps.tile([1, 512], F32, tag="pa")
                    nc.tensor.matmul(ps_s2[:], ones_col[:],
                                     SQo[:, 512 * c:512 * (c + 1)],
                                     start=True, stop=True)
                    nc.vector.tensor_scalar_mul(x2s[0:1, 512 * c:512 * (c + 1)],
                                                ps_s2[:], CS[0:1, 1:2])
                    nc.sync.dma_start(L66[D + 1:D + 2, 512 * c:512 * (c + 1)],
                                      x2s[0:1, 512 * c:512 * (c + 1)])
                    for tt in range(4 * c, 4 * (c + 1)):
                        ps_xe = pa_ps.tile([128, D], F32, tag="pa")
                        nc.tensor.transpose(ps_xe[:], ETo[:, 128 * tt:128 * (tt + 1)],
                                            I128[0:D, 0:D])
                        xe_t = pa_sb.tile([128, D], F32)
                        nc.vector.tensor_copy(xe_t[:], ps_xe[:])
                        nc.sync.dma_start(xe_d[128 * tt:128 * (tt + 1), :], xe_t[:])

            # ---------------- phase B: gumbel + topk ----------------
            with (
                tc.tile_pool(name="ps", bufs=2, space="PSUM") as psp,
                tc.tile_pool(name="sdp", bufs=3) as sdp,
                tc.tile_pool(name="nlp", bufs=4) as nlp,
                tc.tile_pool(name="outp", bufs=3) as outp,
            ):
                for t in range(NT):
                    if t in pref_q:
                        Qt = pref_q[t]
                    else:
                        Qt = qio.tile([128, N], F32, tag="Qt")
                        nc.sync.dma_start(Qt[:], q_d[128 * t:128 * (t + 1), :])
                        # Qt <- ln(q + eps); Qt <- g = ln(-Qt)  (in place)
                        if "ln" not in skip:
                            nc.scalar.activation(Qt[:], Qt[:], AF.Ln,
                                                 bias=eps_ap[:], scale=1.0)
                            nc.scalar.activation(Qt[:], Qt[:], AF.Ln, bias=0.0,
                                                 scale=-1.0)

                    NLt = nlp.tile([128, N], F32)
                    Ct = outp.tile([128, NCAND], F32)
                    CLIt = outp.tile([128, NCAND], U32)
                    V16t = outp.tile([128, 16], F32)
                    P16t = outp.tile([128, 16], U32)
                    Lsl = L66[:, 128 * t:128 * (t + 1)]
                    fine = 4 if t <= 3 else 1   # sub-splits per half (fill)
                    for h in range(2):
                        SDt2 = sdp.tile([128, N // 2], F32, tag="SDt2")
                        psh = psp.tile([128, N // 2], F32)
                        if "mm" not in skip:
                            for c in range(4):
                                ch = 4 * h + c
                                nc.tensor.matmul(psh[:, 512 * c:512 * (c + 1)], Lsl,
                                                 B66[:, 512 * ch:512 * (ch + 1)],
                                                 start=True, stop=True)
                        else:
                            nc.vector.memset(psh[:], 0.0)
                        # ACT evicts raw -s*D; Pool combines with g
                        W2 = N // 2 // fine
                        for f in range(fine):
                            nc.scalar.mul(SDt2[:, W2 * f:W2 * (f + 1)],
                                          psh[:, W2 * f:W2 * (f + 1)], 1.0)
                            nc.gpsimd.tensor_add(
                                out=NLt[:, 2048 * h + W2 * f:2048 * h + W2 * (f + 1)],
                                in0=Qt[:, 2048 * h + W2 * f:2048 * h + W2 * (f + 1)],
                                in1=SDt2[:, W2 * f:W2 * (f + 1)])

                    if "topk" in skip:
                        nc.vector.max(out=V16t[:, 0:8], in_=NLt[:])
                        nc.vector.max(out=V16t[:, 8:16], in_=NLt[:])
                        nc.vector.tensor_copy(P16t[:], V16t[:].bitcast(U32))
                        nc.vector.tensor_copy(CLIt[:], Ct[:].bitcast(U32))
                    else:
                        for c in range(NCH):
                            sl = NLt[:, CHW * c:CHW * (c + 1)]
                            nc.vector.max(out=Ct[:, 8 * c:8 * (c + 1)], in_=sl)
                            nc.vector.max_index(out=CLIt[:, 8 * c:8 * (c + 1)],
                                                in_max=Ct[:, 8 * c:8 * (c + 1)],
                                                in_values=sl)
                        nc.vector.max(out=V16t[:, 0:8], in_=Ct[:])
                        nc.vector.max_index(out=P16t[:, 0:8], in_max=V16t[:, 0:8],
                                            in_values=Ct[:])
                        Cm = outp.tile([128, NCAND], F32)
                        nc.vector.match_replace(out=Cm[:], in_to_replace=V16t[:, 0:8],
                                                in_values=Ct[:], imm_value=NEG_INF)
                        nc.vector.max(out=V16t[:, 8:16], in_=Cm[:])
                        nc.vector.max_index(out=P16t[:, 8:16], in_max=V16t[:, 8:16],
                                            in_values=Cm[:])

                    nc.sync.dma_start(v16_d[128 * t:128 * (t + 1), :], V16t[:])
                    nc.sync.dma_start(p16_d[128 * t:128 * (t + 1), :], P16t[:])
                    nc.sync.dma_start(cli_d[128 * t:128 * (t + 1), :], CLIt[:])
            qio_cm.__exit__(None, None, None)
    nc.finalize()
    return nc


def make_in_maps(x, W, temperature, q):
    x = np.ascontiguousarray(x, np.float32)
    W = np.ascontiguousarray(W, np.float32)
    q = np.ascontiguousarray(q, np.float32)
    s = np.exp(np.clip(np.asarray(temperature, np.float32), -5.0, 5.0))[0]
    consts = np.zeros([128, 2], np.float32)
    consts[:, 0] = np.float32(2.0) * s
    consts[:, 1] = -s
    in_maps = []
    for core in range(2 * B):
        bb, h = core // 2, core % 2
        r0 = h * RPC
        xt = np.ascontiguousarray(x[bb].T)
        in_maps.append({
            "xT_all": xt,
            "xT_own": np.ascontiguousarray(xt[:, r0:r0 + RPC]),
            "w": W,
            "consts": consts,
            "ones_row": np.ones([1, N], np.float32),
            "negs_row": np.full([1, RPC], -s, np.float32),
            "q": np.ascontiguousarray(q[bb, r0:r0 + RPC, :]),
        })
    return in_maps


def postprocess(results):
    """results: list of 8 dicts with x_emb/v16/p16/cli -> (x_emb, edges, logprobs)."""
    x_emb = np.empty([B, N, D], np.float32)
    logprobs = np.empty([B, N, K], np.float32)
    indices = np.empty([B, N, K], np.int32)
    chunk_base = (np.arange(NCAND, dtype=np.int64) // 8) * CHW
    for core in range(2 * B):
        r = results[core]
        bb, h = core // 2, core % 2
        r0 = h * RPC
        x_emb[bb, r0:r0 + RPC] = r["x_emb"]
        cj = r["cli"].astype(np.int64) + chunk_base[None, :]
        j16 = np.take_along_axis(cj, r["p16"].astype(np.int64), axis=1)
        logprobs[bb, r0:r0 + RPC] = r["v16"][:, :K]
        indices[bb, r0:r0 + RPC] = j16[:, :K].astype(np.int32)
    off = (np.arange(B, dtype=np.int32) * N)[:, None, None]
    src = np.broadcast_to(np.arange(N, dtype=np.int32)[None, :, None],
                          (B, N, K)) + off
    tgt = indices + off
    edges = np.stack([src.reshape(-1), tgt.reshape(-1)], axis=0)
    return x_emb, edges, logprobs


_module_cache = {}


def kernel(x, W, temperature, q):
    if "nc" not in _module_cache:
        _module_cache["nc"] = build_module()
    nc = _module_cache["nc"]
    in_maps = make_in_maps(x, W, temperature, q)
    res = run_bass_kernel_spmd(nc, in_maps, list(range(2 * B)))
    return postprocess(res.results)
